# revision 4
# baseline (speedup 1.0000x reference)
"""EntropyGate fused kernel for 8 Trainium2 NeuronCores.

Problem (hardcoded shapes): B=4, S=4096, D=2048, window=8.
  H = entropy of softmax over sliding causal window (8) of token L2 norms of x
  gate_in = [y_ssm | y_attn | H]  (B,S,2D+1)
  h = silu(gate_in @ W1 + b1); g = sigmoid(h @ W2 + b2)
  out = g*y_ssm + (1-g)*y_attn

Sharding: flatten tokens (B*S = 16384) -> 8 shards of 2048 tokens (each shard
lies within one sequence; halo of 7 previous tokens of x for the entropy
window, zeros at sequence starts). Gate MLP weights replicated.

Device layout: feature-major ("transposed") activations so the contraction
dim (features) lands on SBUF partitions. Host supplies y_ssm/y_attn shards
pre-transposed (bf16 for matmul + f32 copy for the final gating); output is
produced transposed [D, tok] and transposed back on host.
"""

import numpy as np
import ml_dtypes

P = 128
D = 2048
TOK = 2048        # tokens per core
HALF = 1024       # token half processed per pass
NT = 512          # psum n-tile (fp32 PSUM bank limit)
MT = 16           # d_out tiles of 128
KC = 32           # 128-row feature chunks of [yT_ssm; yT_attn]
K2 = 16           # contraction chunks for mm2
WIN = 8
EXT = TOK + WIN - 1   # 2055
N_CORES = 8
B, S = 4, 4096

_BF16 = ml_dtypes.bfloat16
_NC_CACHE = {}


def _build_nc():
    import concourse.bass as bass
    import concourse.tile as tile
    import concourse.mybir as mybir
    from concourse import bacc

    f32 = mybir.dt.float32
    bf16 = mybir.dt.bfloat16
    AF = mybir.ActivationFunctionType
    AX = mybir.AxisListType

    nc = bacc.Bacc("TRN2", target_bir_lowering=False, debug=False, num_devices=1)

    yt16 = nc.dram_tensor("yt16", [2 * D, TOK], bf16, kind="ExternalInput")
    yf = nc.dram_tensor("yf", [2 * D, TOK], f32, kind="ExternalInput")
    xh = nc.dram_tensor("xh", [EXT, D], bf16, kind="ExternalInput")
    w1 = nc.dram_tensor("w1", [2 * D + 1, D], bf16, kind="ExternalInput")
    w2 = nc.dram_tensor("w2", [D, D], bf16, kind="ExternalInput")
    b1v = nc.dram_tensor("b1v", [D], f32, kind="ExternalInput")
    b2v = nc.dram_tensor("b2v", [D], f32, kind="ExternalInput")
    outT = nc.dram_tensor("outT", [D, TOK], f32, kind="ExternalOutput")
    m_dram = nc.dram_tensor("m_scr", [17 * P], f32, kind="Internal")
    h_dram = nc.dram_tensor("h_scr", [TOK], bf16, kind="Internal")

    with tile.TileContext(nc) as tc:
        from contextlib import ExitStack
        with ExitStack() as ctx:
            ent = ctx.enter_context(tc.tile_pool(name="ent", bufs=2))
            smol = ctx.enter_context(tc.tile_pool(name="smol", bufs=2))
            const = ctx.enter_context(tc.tile_pool(name="const", bufs=1))
            gate = ctx.enter_context(tc.tile_pool(name="gate", bufs=33))
            htp = ctx.enter_context(tc.tile_pool(name="htp", bufs=17))
            w1p = ctx.enter_context(tc.tile_pool(name="w1p", bufs=6))
            w2p = ctx.enter_context(tc.tile_pool(name="w2p", bufs=6))
            yfp = ctx.enter_context(tc.tile_pool(name="yfp", bufs=4))
            gp = ctx.enter_context(tc.tile_pool(name="gp", bufs=4))
            tp = ctx.enter_context(tc.tile_pool(name="tp", bufs=3))
            op = ctx.enter_context(tc.tile_pool(name="op", bufs=4))
            ps = ctx.enter_context(tc.tile_pool(name="ps", bufs=8, space="PSUM"))

            # ---- biases (per-partition columns: b[p, m] = b[m*128 + p]) ----
            b1sb = const.tile([P, MT], f32)
            nc.sync.dma_start(b1sb[:], bass.AP(b1v, 0, [[1, P], [P, MT]]))
            b2sb = const.tile([P, MT], f32)
            nc.sync.dma_start(b2sb[:], bass.AP(b2v, 0, [[1, P], [P, MT]]))

            # ---- entropy: token norms m = ||x||, 17 tiles of 128 ext-tokens ----
            mcol = const.tile([P, 17], f32)
            nc.vector.memset(mcol[:], 1.0)
            for i in range(17):
                rows = P if i < 16 else EXT - 16 * P
                xt = ent.tile([P, D], bf16)
                nc.sync.dma_start(xt[:rows, :], xh.ap()[i * P:i * P + rows, :])
                nc.scalar.activation(
                    xt[:rows, :], xt[:rows, :], AF.Square,
                    accum_out=mcol[:rows, i:i + 1],
                )
            # sqrt + one Newton step (ACT sqrt table is low-precision)
            y0 = smol.tile([P, 17], f32)
            nc.scalar.sqrt(y0[:], mcol[:])
            y0e = smol.tile([P, 17], f32)
            nc.vector.tensor_scalar_add(y0e[:], y0[:], 1e-30)
            rcp = smol.tile([P, 17], f32)
            nc.vector.reciprocal(rcp[:], y0e[:])
            qt = smol.tile([P, 17], f32)
            nc.vector.tensor_mul(qt[:], mcol[:], rcp[:])
            msum = smol.tile([P, 17], f32)
            nc.vector.tensor_add(msum[:], y0[:], qt[:])
            mf = smol.tile([P, 17], f32)
            nc.scalar.mul(mf[:], msum[:], 0.5)
            nc.sync.dma_start(bass.AP(m_dram, 0, [[1, P], [P, 17]]), mf[:])

            # ---- windows: wt[p, f, j] = m_ext[p*16 + f + j]  (token t=p*16+f) ----
            negC = const.tile([P, 1], f32)
            nc.vector.memset(negC[:], -45.0)
            wt = smol.tile([P, 16, WIN], f32)
            nc.sync.dma_start(wt[:], bass.AP(m_dram, 0, [[16, P], [1, 16], [1, WIN]]))
            et = smol.tile([P, 16, WIN], f32)
            nc.scalar.activation(et[:], wt[:], AF.Exp, bias=negC[:])
            pw = smol.tile([P, 16, WIN], f32)
            nc.vector.tensor_mul(pw[:], et[:], wt[:])
            S_ = smol.tile([P, 16], f32)
            nc.vector.reduce_sum(S_[:], et[:], axis=AX.X)
            T_ = smol.tile([P, 16], f32)
            nc.vector.reduce_sum(T_[:], pw[:], axis=AX.X)
            R_ = smol.tile([P, 16], f32)
            nc.vector.reciprocal(R_[:], S_[:])
            L_ = smol.tile([P, 16], f32)
            nc.scalar.activation(L_[:], S_[:], AF.Ln)
            U_ = smol.tile([P, 16], f32)
            nc.vector.tensor_mul(U_[:], T_[:], R_[:])
            V_ = smol.tile([P, 16], f32)
            nc.vector.tensor_sub(V_[:], L_[:], U_[:])
            Hb = smol.tile([P, 16], bf16)
            nc.vector.tensor_scalar(
                Hb[:], V_[:], 45.0, 1.4426950408889634,
                op0=mybir.AluOpType.add, op1=mybir.AluOpType.mult,
            )
            nc.sync.dma_start(bass.AP(h_dram, 0, [[16, P], [1, 16]]), Hb[:])

            # ---- main: two token-halves ----
            for h in range(2):
                csl = slice(h * HALF, (h + 1) * HALF)
                gts = []
                for k in range(KC):
                    gt = gate.tile([P, HALF], bf16)
                    nc.sync.dma_start(gt[:], yt16.ap()[k * P:(k + 1) * P, csl])
                    gts.append(gt)
                hrow = const.tile([1, HALF], bf16)
                nc.sync.dma_start(
                    hrow[:], bass.AP(h_dram, h * HALF, [[HALF, 1], [1, HALF]])
                )

                hts = [htp.tile([P, HALF], bf16, name="ht", tag="ht") for _ in range(MT)]

                # mm1: hT[m, tok] = silu(sum_k W1[k,m].T @ gateT[k,tok] + b1)
                for mg in range(4):
                    pts = [[ps.tile([P, NT], f32, name="pt1", tag="pt") for _ in range(2)] for _ in range(4)]
                    for k in range(KC):
                        wtile = w1p.tile([P, 4 * P], bf16)
                        nc.sync.dma_start(
                            wtile[:], w1.ap()[k * P:(k + 1) * P,
                                              mg * 512:(mg + 1) * 512]
                        )
                        for mi in range(4):
                            for n in range(2):
                                nc.tensor.matmul(
                                    pts[mi][n][:],
                                    wtile[:, mi * P:(mi + 1) * P],
                                    gts[k][:, n * NT:(n + 1) * NT],
                                    start=(k == 0), stop=False,
                                )
                    wH = w1p.tile([1, 4 * P], bf16)
                    nc.sync.dma_start(
                        wH[:], w1.ap()[2 * D:2 * D + 1, mg * 512:(mg + 1) * 512]
                    )
                    for mi in range(4):
                        m = mg * 4 + mi
                        for n in range(2):
                            nc.tensor.matmul(
                                pts[mi][n][:],
                                wH[:, mi * P:(mi + 1) * P],
                                hrow[:, n * NT:(n + 1) * NT],
                                start=False, stop=True,
                            )
                            nc.scalar.activation(
                                hts[m][:, n * NT:(n + 1) * NT], pts[mi][n][:],
                                AF.Silu, bias=b1sb[:, m:m + 1],
                            )

                # mm2 + sigmoid + gating
                for eg in range(4):
                    pts2 = [[ps.tile([P, NT], f32, name="pt2", tag="pt") for _ in range(2)] for _ in range(4)]
                    for k2 in range(K2):
                        wtile2 = w2p.tile([P, 4 * P], bf16)
                        nc.sync.dma_start(
                            wtile2[:], w2.ap()[k2 * P:(k2 + 1) * P,
                                               eg * 512:(eg + 1) * 512]
                        )
                        for ei in range(4):
                            for n in range(2):
                                nc.tensor.matmul(
                                    pts2[ei][n][:],
                                    wtile2[:, ei * P:(ei + 1) * P],
                                    hts[k2][:, n * NT:(n + 1) * NT],
                                    start=(k2 == 0), stop=(k2 == K2 - 1),
                                )
                    for ei in range(4):
                        e = eg * 4 + ei
                        ysf = yfp.tile([P, HALF], f32)
                        nc.sync.dma_start(ysf[:], yf.ap()[e * P:(e + 1) * P, csl])
                        yaf = yfp.tile([P, HALF], f32)
                        nc.sync.dma_start(
                            yaf[:], yf.ap()[D + e * P:D + (e + 1) * P, csl]
                        )
                        for n in range(2):
                            nsl = slice(n * NT, (n + 1) * NT)
                            g = gp.tile([P, NT], f32)
                            nc.scalar.activation(
                                g[:], pts2[ei][n][:], AF.Sigmoid,
                                bias=b2sb[:, e:e + 1],
                            )
                            dsub = tp.tile([P, NT], f32)
                            nc.vector.tensor_sub(dsub[:], ysf[:, nsl], yaf[:, nsl])
                            prod = tp.tile([P, NT], f32)
                            nc.vector.tensor_mul(prod[:], g[:], dsub[:])
                            ot = op.tile([P, NT], f32)
                            nc.vector.tensor_add(ot[:], prod[:], yaf[:, nsl])
                            nc.sync.dma_start(
                                outT.ap()[e * P:(e + 1) * P,
                                          h * HALF + n * NT:h * HALF + (n + 1) * NT],
                                ot[:],
                            )
    nc.finalize()
    return nc


def _get_nc():
    if "nc" not in _NC_CACHE:
        _NC_CACHE["nc"] = _build_nc()
    return _NC_CACHE["nc"]


def _make_in_maps(y_ssm, y_attn, x, W1, b1, W2, b2):
    ys = np.ascontiguousarray(np.asarray(y_ssm, np.float32).reshape(-1, D))
    ya = np.ascontiguousarray(np.asarray(y_attn, np.float32).reshape(-1, D))
    xs = np.ascontiguousarray(np.asarray(x, np.float32).reshape(-1, D))
    w1_bf = np.asarray(W1, np.float32).astype(_BF16)
    w2_bf = np.asarray(W2, np.float32).astype(_BF16)
    b1f = np.ascontiguousarray(np.asarray(b1, np.float32))
    b2f = np.ascontiguousarray(np.asarray(b2, np.float32))

    in_maps = []
    for c in range(N_CORES):
        t0 = c * TOK
        ysT = np.ascontiguousarray(ys[t0:t0 + TOK].T)   # (D, TOK) f32
        yaT = np.ascontiguousarray(ya[t0:t0 + TOK].T)
        yt16 = np.empty((2 * D, TOK), _BF16)
        yt16[:D] = ysT
        yt16[D:] = yaT
        yfc = np.empty((2 * D, TOK), np.float32)
        yfc[:D] = ysT
        yfc[D:] = yaT
        xe = np.zeros((EXT, D), np.float32)
        if t0 % S != 0:
            xe[:WIN - 1] = xs[t0 - (WIN - 1):t0]
        xe[WIN - 1:] = xs[t0:t0 + TOK]
        in_maps.append({
            "yt16": yt16,
            "yf": yfc,
            "xh": xe.astype(_BF16),
            "w1": w1_bf,
            "w2": w2_bf,
            "b1v": b1f,
            "b2v": b2f,
        })
    return in_maps


def _run(in_maps, trace=False):
    from concourse.bass_utils import run_bass_kernel_spmd
    nc = _get_nc()
    return run_bass_kernel_spmd(
        nc, in_maps, core_ids=list(range(N_CORES)), trace=trace
    )


def kernel(y_ssm, y_attn, x, W1, b1, W2, b2):
    in_maps = _make_in_maps(y_ssm, y_attn, x, W1, b1, W2, b2)
    res = _run(in_maps, trace=False)
    shards = [np.ascontiguousarray(r["outT"].T) for r in res.results]  # (TOK, D)
    full = np.concatenate(shards, axis=0)  # (16384, D)
    return full.reshape(B, S, D).astype(np.float32)
